# revision 17
# baseline (speedup 1.0000x reference)
"""Adaptive embedding (4-cluster masked embedding + projection) on 8 trn2 cores.

Sharding: tokens are globally sorted by cluster and dealt round-robin to the
8 NeuronCores, so per-core per-cluster counts are equal (+-1) and tile padding
is minimal (19 128-token tiles per core); the tables are replicated.

Host does ROUTING only; the device gathers rows with one `indirect_dma_start`
per 128-token tile (int32 indices, one row per partition), PE-transposes each
gathered tile, projects clusters 1-3 in bf16 on the PE, and writes bf16
output in a partition-major layout the host un-transposes and upcasts.
Cluster 0 needs no projection and streams gathered rows straight to DRAM.

All weights ship as one pre-packed [128, 4096] bf16 blob, consumed directly
by the PE (no staging casts). The sqrt(D_PROJ)=32 output scale is an exact
power of two, folded into the tables / projection matrices before the bf16
cast on the host.

Numerics: device data path is bf16 (inputs rounded once on host); matmul
accumulation is fp32 in PSUM. Worst-case elementwise error ~2^-8 relative,
far inside the 2e-2 gate.
"""

import os

import numpy as np

CUTOFFS = (0, 20000, 40000, 200000, 267735)
D_PROJ = 1024
N_CORES = 8
P = 128

VOCABS = (20000, 20000, 160000, 67735)
DES = (1024, 256, 64, 16)

_BUILD_CACHE = {}
LAST_RESULT = None  # BassKernelResults of the most recent run (for profiling)


def _cfg():
    return dict(
        evac=os.environ.get("KERNEL_EVAC", "x"),
        warm_mm=int(os.environ.get("KERNEL_WARM_MM", "6")),
        store_batch=int(os.environ.get("KERNEL_STORE_BATCH", "1")),
    )


def _build(caps, cfg):
    """SPMD Bass program. caps: 128-token tiles per cluster (identical on
    every core - tokens are dealt round-robin from the global sort)."""
    import concourse.bass as bass
    import concourse.bacc as bacc
    import concourse.tile as tile
    from concourse import mybir
    from concourse.masks import make_identity

    f32 = mybir.dt.float32
    bf16 = mybir.dt.bfloat16
    i32 = mybir.dt.int32
    ntsum = sum(caps)
    col0 = [0, caps[0], caps[0] + caps[1], caps[0] + caps[1] + caps[2]]

    nc = bacc.Bacc("TRN2", target_bir_lowering=False)
    emb = [
        nc.dram_tensor(f"emb{i}", [VOCABS[i], DES[i]], bf16, kind="ExternalInput")
        for i in range(4)
    ]
    # weights blob [128, 4096]: W1[0:128] | W1[128:256] | W2 (rows 0:64) | W3
    # (rows 0:16)
    wts = nc.dram_tensor("wts", [P, 4 * D_PROJ], bf16, kind="ExternalInput")
    idx32 = nc.dram_tensor("idx32", [P, ntsum], i32, kind="ExternalInput")
    out = [
        nc.dram_tensor(f"out{i}", [P, caps[i], D_PROJ], bf16, kind="ExternalOutput")
        for i in range(4)
    ]

    with tile.TileContext(nc) as tc:
        with (
            tc.tile_pool(name="const", bufs=1) as cpool,
            tc.tile_pool(name="xt", bufs=4) as xtpool,
            tc.tile_pool(name="tpsum", bufs=2, space="PSUM") as tppool,
            tc.tile_pool(name="mpsum", bufs=3, space="PSUM") as mpool,
        ):
            idxt = cpool.tile([P, ntsum], i32, name="idxt")
            nc.sync.dma_start(out=idxt[:], in_=idx32[:])

            wt = cpool.tile([P, 4 * D_PROJ], bf16, name="wt")
            nc.scalar.dma_start(out=wt[:], in_=wts[:])

            ident = cpool.tile([P, P], bf16, name="ident")
            make_identity(nc, ident)

            # PE warm-up (HAM clock-gate) while the first gathers run
            nwarm = cfg["warm_mm"]
            if nwarm:
                warm = cpool.tile([P, 512 + P], bf16, name="warm")
                nc.gpsimd.memset(warm[:], 0.0)
                wps = mpool.tile([P, D_PROJ], f32, tag="ps", name="warm_ps")
                for w in range(nwarm):
                    nc.tensor.matmul(
                        wps[:, 0:512], warm[:, 512 : 512 + P], warm[:, 0:512],
                        start=True, stop=True,
                    )

            g = [
                cpool.tile([P, caps[i], DES[i]], bf16, name=f"g{i}")
                for i in range(4)
            ]

            def gather(i, t):
                nc.gpsimd.indirect_dma_start(
                    out=g[i][:, t, :],
                    out_offset=None,
                    in_=emb[i][:],
                    in_offset=bass.IndirectOffsetOnAxis(
                        ap=idxt[:, col0[i] + t : col0[i] + t + 1], axis=0
                    ),
                )

            # issue order: two c2 tiles to prime the PE chain, cluster 0 for
            # early store flow, then weave the rest (c2 paced, c1/c3 between)
            gorder = [(2, 0), (2, 1)]
            rest = [(2, t) for t in range(2, caps[2])]
            mix = [(1, t) for t in range(caps[1])] + [(3, t) for t in range(caps[3])]
            step = max(1, len(rest) // max(1, len(mix)))
            merged = []
            mi = 0
            for j, it in enumerate(rest):
                merged.append(it)
                if (j + 1) % step == 0 and mi < len(mix):
                    merged.append(mix[mi])
                    mi += 1
            merged += mix[mi:]
            gorder += merged
            gorder += [(0, t) for t in range(caps[0])]
            for i, t in gorder:
                gather(i, t)

            # cluster 0: straight copy to DRAM (p-major layout = same layout)
            nc.sync.dma_start(out=out[0][:, :, :], in_=g[0][:, :, :])

            stage = {
                i: cpool.tile([P, caps[i], D_PROJ], bf16, name=f"stage{i}")
                for i in (1, 2, 3)
            }

            evac_pat = cfg["evac"]
            evac_state = [0]

            def evac(i, t, ps):
                e = evac_pat[evac_state[0] % len(evac_pat)]
                evac_state[0] += 1
                dst = stage[i][:, t, :]
                if e == "v":
                    nc.vector.tensor_copy(out=dst, in_=ps[:])
                elif e == "s":
                    nc.scalar.copy(out=dst, in_=ps[:])
                else:  # split across both engines
                    nc.vector.tensor_copy(out=dst[:, 0:512], in_=ps[:, 0:512])
                    nc.scalar.copy(out=dst[:, 512:1024], in_=ps[:, 512:1024])

            def store(i, t0, t1):
                nc.sync.dma_start(
                    out=out[i][:, t0:t1, :], in_=stage[i][:, t0:t1, :]
                )

            sb = cfg["store_batch"]
            pend = {1: 0, 2: 0, 3: 0}

            # wcol: weight-blob column base; c1 contracts in 2 K-chunks
            def project_tile(i, t):
                de = DES[i]
                nch = 2 if i == 1 else 1
                wcol = {1: 0, 2: 2 * D_PROJ, 3: 3 * D_PROJ}[i]
                ps = mpool.tile([P, D_PROJ], f32, tag="ps", name=f"ps{i}_{t}")
                for k in range(nch):
                    w = min(P, de)
                    tp = tppool.tile([P, P], bf16, tag="tp", name=f"tp{i}_{t}_{k}")
                    nc.tensor.transpose(
                        out=tp[:w, :],
                        in_=g[i][:, t, k * P : k * P + w],
                        identity=ident[:],
                    )
                    xt = xtpool.tile([P, P], bf16, tag="xt", name=f"xt{i}_{t}_{k}")
                    nc.vector.tensor_copy(out=xt[:w, :], in_=tp[:w, :])
                    for n in range(2):
                        col = wcol + k * D_PROJ + n * 512
                        nc.tensor.matmul(
                            ps[:, n * 512 : (n + 1) * 512],
                            xt[0:w, :],
                            wt[0:w, col : col + 512],
                            start=(k == 0),
                            stop=(k == nch - 1),
                        )
                evac(i, t, ps)
                if t + 1 - pend[i] >= sb or t == caps[i] - 1:
                    store(i, pend[i], t + 1)
                    pend[i] = t + 1

            for i, t in gorder:
                if i != 0:
                    project_tile(i, t)

    nc.compile()
    return nc


def kernel(tokens, emb0, emb1, emb2, emb3, proj1, proj2, proj3):
    global LAST_RESULT
    import ml_dtypes
    from concourse.bass_utils import run_bass_kernel_spmd

    bf16 = ml_dtypes.bfloat16
    toks = np.asarray(tokens).astype(np.int64, copy=False)
    nb_, ns = toks.shape
    assert nb_ == N_CORES and ns % P == 0

    # fold sqrt(1024)=32 (exact in bf16) and round tables once on the host
    scale = np.float32(32.0)
    embs = [
        np.ascontiguousarray((np.asarray(emb0, dtype=np.float32) * scale).astype(bf16)),
        np.ascontiguousarray(np.asarray(emb1, dtype=np.float32).astype(bf16)),
        np.ascontiguousarray(np.asarray(emb2, dtype=np.float32).astype(bf16)),
        np.ascontiguousarray(np.asarray(emb3, dtype=np.float32).astype(bf16)),
    ]
    w1 = (np.asarray(proj1, dtype=np.float32) * scale).astype(bf16)
    w2z = np.zeros((P, D_PROJ), bf16)
    w2z[:64] = (np.asarray(proj2, dtype=np.float32) * scale).astype(bf16)
    w3z = np.zeros((P, D_PROJ), bf16)
    w3z[:16] = (np.asarray(proj3, dtype=np.float32) * scale).astype(bf16)
    wts = np.ascontiguousarray(
        np.concatenate([w1[0:P], w1[P : 2 * P], w2z, w3z], axis=1)
    )

    # ---- global routing: cluster per token, global sort, round-robin deal
    ft = toks.reshape(-1)
    cuts = np.asarray(CUTOFFS, dtype=np.int64)
    fcl = np.searchsorted(cuts[1:-1], ft, side="right")
    sizes = np.asarray(VOCABS, dtype=np.int64)
    floc = np.clip(ft - cuts[fcl], 0, sizes[fcl] - 1)

    order_g = np.argsort(fcl, kind="stable")
    nbt = np.bincount(fcl, minlength=4)
    starts = np.concatenate([[0], np.cumsum(nbt)])
    percore = [-(-int(nbt[b]) // N_CORES) for b in range(4)]
    caps = tuple(int(max(1, -(-percore[b] // P))) for b in range(4))

    cfg = _cfg()
    key = (caps, tuple(sorted(cfg.items())))
    if key not in _BUILD_CACHE:
        _BUILD_CACHE[key] = _build(caps, cfg)
    nc = _BUILD_CACHE[key]

    in_maps = []
    for c in range(N_CORES):
        m = {f"emb{i}": embs[i] for i in range(4)}
        m["wts"] = wts
        cols = []
        for b in range(4):
            gidx = order_g[starts[b] : starts[b + 1]][c::N_CORES]
            padded = np.zeros(caps[b] * P, np.int32)
            padded[: len(gidx)] = floc[gidx]
            # device layout: idx[p, t] = local position t*128 + p
            cols.append(padded.reshape(caps[b], P).T)
        m["idx32"] = np.ascontiguousarray(np.concatenate(cols, axis=1))
        in_maps.append(m)

    res = run_bass_kernel_spmd(nc, in_maps, core_ids=list(range(N_CORES)))
    LAST_RESULT = res

    flat = np.empty((N_CORES * ns, D_PROJ), np.float32)
    for c in range(N_CORES):
        for b in range(4):
            gidx = order_g[starts[b] : starts[b + 1]][c::N_CORES]
            if len(gidx) == 0:
                continue
            o = np.asarray(res.results[c][f"out{b}"])  # [128, caps_b, 1024]
            rows = o.transpose(1, 0, 2).reshape(caps[b] * P, D_PROJ)
            flat[gidx] = rows[: len(gidx)].astype(np.float32)
    return flat.reshape(N_CORES, ns, D_PROJ)


# revision 18
# speedup vs baseline: 1.0368x; 1.0368x over previous
"""Adaptive embedding (4-cluster masked embedding + projection) on 8 trn2 cores.

Sharding: tokens are globally sorted by cluster and dealt round-robin to the
8 NeuronCores, so per-core per-cluster counts are equal (+-1) and tile padding
is minimal (19 128-token tiles per core); the tables are replicated.

Host does ROUTING only; the device gathers rows with one `indirect_dma_start`
per 128-token tile (int32 indices, one row per partition), PE-transposes each
gathered tile, projects clusters 1-3 in bf16 on the PE, and writes bf16
output in a partition-major layout the host un-transposes and upcasts.
Cluster 0 needs no projection and streams gathered rows straight to DRAM.

All weights ship as one pre-packed [128, 4096] bf16 blob, consumed directly
by the PE (no staging casts). The sqrt(D_PROJ)=32 output scale is an exact
power of two, folded into the tables / projection matrices before the bf16
cast on the host.

Numerics: device data path is bf16 (inputs rounded once on host); matmul
accumulation is fp32 in PSUM. Worst-case elementwise error ~2^-8 relative,
far inside the 2e-2 gate.
"""

import os

import numpy as np

CUTOFFS = (0, 20000, 40000, 200000, 267735)
D_PROJ = 1024
N_CORES = 8
P = 128

VOCABS = (20000, 20000, 160000, 67735)
DES = (1024, 256, 64, 16)

_BUILD_CACHE = {}
LAST_RESULT = None  # BassKernelResults of the most recent run (for profiling)


def _cfg():
    return dict(
        evac=os.environ.get("KERNEL_EVAC", "vs"),
        warm_mm=int(os.environ.get("KERNEL_WARM_MM", "6")),
        store_batch=int(os.environ.get("KERNEL_STORE_BATCH", "2")),
    )


def _build(caps, cfg):
    """SPMD Bass program. caps: 128-token tiles per cluster (identical on
    every core - tokens are dealt round-robin from the global sort)."""
    import concourse.bass as bass
    import concourse.bacc as bacc
    import concourse.tile as tile
    from concourse import mybir
    from concourse.masks import make_identity

    f32 = mybir.dt.float32
    bf16 = mybir.dt.bfloat16
    i32 = mybir.dt.int32
    ntsum = sum(caps)
    col0 = [0, caps[0], caps[0] + caps[1], caps[0] + caps[1] + caps[2]]

    nc = bacc.Bacc("TRN2", target_bir_lowering=False)
    emb = [
        nc.dram_tensor(f"emb{i}", [VOCABS[i], DES[i]], bf16, kind="ExternalInput")
        for i in range(4)
    ]
    # weights blob [128, 4096]: W1[0:128] | W1[128:256] | W2 (rows 0:64) | W3
    # (rows 0:16)
    wts = nc.dram_tensor("wts", [P, 4 * D_PROJ], bf16, kind="ExternalInput")
    idx32 = nc.dram_tensor("idx32", [P, ntsum], i32, kind="ExternalInput")
    out = [
        nc.dram_tensor(f"out{i}", [P, caps[i], D_PROJ], bf16, kind="ExternalOutput")
        for i in range(4)
    ]

    with tile.TileContext(nc) as tc:
        with (
            tc.tile_pool(name="const", bufs=1) as cpool,
            tc.tile_pool(name="xt", bufs=4) as xtpool,
            tc.tile_pool(name="tpsum", bufs=2, space="PSUM") as tppool,
            tc.tile_pool(name="mpsum", bufs=3, space="PSUM") as mpool,
        ):
            idxt = cpool.tile([P, ntsum], i32, name="idxt")
            nc.sync.dma_start(out=idxt[:], in_=idx32[:])

            wt = cpool.tile([P, 4 * D_PROJ], bf16, name="wt")
            nc.scalar.dma_start(out=wt[:], in_=wts[:])

            ident = cpool.tile([P, P], bf16, name="ident")
            make_identity(nc, ident)

            # PE warm-up (HAM clock-gate) while the first gathers run
            nwarm = cfg["warm_mm"]
            if nwarm:
                warm = cpool.tile([P, 512 + P], bf16, name="warm")
                nc.gpsimd.memset(warm[:], 0.0)
                wps = mpool.tile([P, D_PROJ], f32, tag="ps", name="warm_ps")
                for w in range(nwarm):
                    nc.tensor.matmul(
                        wps[:, 0:512], warm[:, 512 : 512 + P], warm[:, 0:512],
                        start=True, stop=True,
                    )

            g = [
                cpool.tile([P, caps[i], DES[i]], bf16, name=f"g{i}")
                for i in range(4)
            ]

            def gather(i, t):
                nc.gpsimd.indirect_dma_start(
                    out=g[i][:, t, :],
                    out_offset=None,
                    in_=emb[i][:],
                    in_offset=bass.IndirectOffsetOnAxis(
                        ap=idxt[:, col0[i] + t : col0[i] + t + 1], axis=0
                    ),
                )

            # issue order: two c2 tiles to prime the PE chain, cluster 0 for
            # early store flow, then weave the rest (c2 paced, c1/c3 between)
            gorder = [(2, 0), (2, 1)]
            gorder += [(0, t) for t in range(caps[0])]
            rest = [(2, t) for t in range(2, caps[2])]
            mix = [(1, t) for t in range(caps[1])] + [(3, t) for t in range(caps[3])]
            step = max(1, len(rest) // max(1, len(mix)))
            merged = []
            mi = 0
            for j, it in enumerate(rest):
                merged.append(it)
                if (j + 1) % step == 0 and mi < len(mix):
                    merged.append(mix[mi])
                    mi += 1
            merged += mix[mi:]
            gorder += merged
            for i, t in gorder:
                gather(i, t)

            # cluster 0: straight copy to DRAM (p-major layout = same layout)
            nc.sync.dma_start(out=out[0][:, :, :], in_=g[0][:, :, :])

            stage = {
                i: cpool.tile([P, caps[i], D_PROJ], bf16, name=f"stage{i}")
                for i in (1, 2, 3)
            }

            evac_pat = cfg["evac"]
            evac_state = [0]

            def evac(i, t, ps):
                e = evac_pat[evac_state[0] % len(evac_pat)]
                evac_state[0] += 1
                dst = stage[i][:, t, :]
                if e == "v":
                    nc.vector.tensor_copy(out=dst, in_=ps[:])
                elif e == "s":
                    nc.scalar.copy(out=dst, in_=ps[:])
                else:  # split across both engines
                    nc.vector.tensor_copy(out=dst[:, 0:512], in_=ps[:, 0:512])
                    nc.scalar.copy(out=dst[:, 512:1024], in_=ps[:, 512:1024])

            def store(i, t0, t1):
                nc.sync.dma_start(
                    out=out[i][:, t0:t1, :], in_=stage[i][:, t0:t1, :]
                )

            sb = cfg["store_batch"]
            pend = {1: 0, 2: 0, 3: 0}

            # wcol: weight-blob column base; c1 contracts in 2 K-chunks
            def project_tile(i, t):
                de = DES[i]
                nch = 2 if i == 1 else 1
                wcol = {1: 0, 2: 2 * D_PROJ, 3: 3 * D_PROJ}[i]
                ps = mpool.tile([P, D_PROJ], f32, tag="ps", name=f"ps{i}_{t}")
                for k in range(nch):
                    w = min(P, de)
                    tp = tppool.tile([P, P], bf16, tag="tp", name=f"tp{i}_{t}_{k}")
                    nc.tensor.transpose(
                        out=tp[:w, :],
                        in_=g[i][:, t, k * P : k * P + w],
                        identity=ident[:],
                    )
                    xt = xtpool.tile([P, P], bf16, tag="xt", name=f"xt{i}_{t}_{k}")
                    nc.vector.tensor_copy(out=xt[:w, :], in_=tp[:w, :])
                    for n in range(2):
                        col = wcol + k * D_PROJ + n * 512
                        nc.tensor.matmul(
                            ps[:, n * 512 : (n + 1) * 512],
                            xt[0:w, :],
                            wt[0:w, col : col + 512],
                            start=(k == 0),
                            stop=(k == nch - 1),
                        )
                evac(i, t, ps)
                if t + 1 - pend[i] >= sb or t == caps[i] - 1:
                    store(i, pend[i], t + 1)
                    pend[i] = t + 1

            for i, t in gorder:
                if i != 0:
                    project_tile(i, t)

    nc.compile()
    return nc


def kernel(tokens, emb0, emb1, emb2, emb3, proj1, proj2, proj3):
    global LAST_RESULT
    import ml_dtypes
    from concourse.bass_utils import run_bass_kernel_spmd

    bf16 = ml_dtypes.bfloat16
    toks = np.asarray(tokens).astype(np.int64, copy=False)
    nb_, ns = toks.shape
    assert nb_ == N_CORES and ns % P == 0

    # fold sqrt(1024)=32 (exact in bf16) and round tables once on the host
    scale = np.float32(32.0)
    embs = [
        np.ascontiguousarray((np.asarray(emb0, dtype=np.float32) * scale).astype(bf16)),
        np.ascontiguousarray(np.asarray(emb1, dtype=np.float32).astype(bf16)),
        np.ascontiguousarray(np.asarray(emb2, dtype=np.float32).astype(bf16)),
        np.ascontiguousarray(np.asarray(emb3, dtype=np.float32).astype(bf16)),
    ]
    w1 = (np.asarray(proj1, dtype=np.float32) * scale).astype(bf16)
    w2z = np.zeros((P, D_PROJ), bf16)
    w2z[:64] = (np.asarray(proj2, dtype=np.float32) * scale).astype(bf16)
    w3z = np.zeros((P, D_PROJ), bf16)
    w3z[:16] = (np.asarray(proj3, dtype=np.float32) * scale).astype(bf16)
    wts = np.ascontiguousarray(
        np.concatenate([w1[0:P], w1[P : 2 * P], w2z, w3z], axis=1)
    )

    # ---- global routing: cluster per token, global sort, round-robin deal
    ft = toks.reshape(-1)
    cuts = np.asarray(CUTOFFS, dtype=np.int64)
    fcl = np.searchsorted(cuts[1:-1], ft, side="right")
    sizes = np.asarray(VOCABS, dtype=np.int64)
    floc = np.clip(ft - cuts[fcl], 0, sizes[fcl] - 1)

    order_g = np.argsort(fcl, kind="stable")
    nbt = np.bincount(fcl, minlength=4)
    starts = np.concatenate([[0], np.cumsum(nbt)])
    percore = [-(-int(nbt[b]) // N_CORES) for b in range(4)]
    caps = tuple(int(max(1, -(-percore[b] // P))) for b in range(4))

    cfg = _cfg()
    key = (caps, tuple(sorted(cfg.items())))
    if key not in _BUILD_CACHE:
        _BUILD_CACHE[key] = _build(caps, cfg)
    nc = _BUILD_CACHE[key]

    in_maps = []
    for c in range(N_CORES):
        m = {f"emb{i}": embs[i] for i in range(4)}
        m["wts"] = wts
        cols = []
        for b in range(4):
            gidx = order_g[starts[b] : starts[b + 1]][c::N_CORES]
            padded = np.zeros(caps[b] * P, np.int32)
            padded[: len(gidx)] = floc[gidx]
            # device layout: idx[p, t] = local position t*128 + p
            cols.append(padded.reshape(caps[b], P).T)
        m["idx32"] = np.ascontiguousarray(np.concatenate(cols, axis=1))
        in_maps.append(m)

    res = run_bass_kernel_spmd(nc, in_maps, core_ids=list(range(N_CORES)))
    LAST_RESULT = res

    flat = np.empty((N_CORES * ns, D_PROJ), np.float32)
    for c in range(N_CORES):
        for b in range(4):
            gidx = order_g[starts[b] : starts[b + 1]][c::N_CORES]
            if len(gidx) == 0:
                continue
            o = np.asarray(res.results[c][f"out{b}"])  # [128, caps_b, 1024]
            rows = o.transpose(1, 0, 2).reshape(caps[b] * P, D_PROJ)
            flat[gidx] = rows[: len(gidx)].astype(np.float32)
    return flat.reshape(N_CORES, ns, D_PROJ)
